# revision 23
# baseline (speedup 1.0000x reference)
"""LIF neuron step on 8 Trainium2 NeuronCores.

Math (reference):
    I_raw   = g @ w                       # [N] vec-mat product, w is [N, N]
    I       = sigmoid(12/N * I_raw) + 0.9 * x_in
    v_next  = v + (E_L - v + I * (30 - E_L)) / tau_m
    out     = sigmoid(v_next - 30)

Everything after the matvec is affine in I_sig = sigmoid(12/N * I_raw):
    out = sigmoid(B * I_sig + D)
    B   = (30 - E_L) / tau_m
    D   = v + (E_L - v)/tau_m - 30 + 0.9 * x_in * B
B and D are tiny per-neuron vectors, computed on the host.

Sharding: w is split column-wise (output-neuron dim) into 8 shards of
[8192, 1024]; g is replicated. Each core computes its 1024 outputs fully
locally; host concatenates.

Design (v2, ~2.5x over the fp16 w-stationary version):
  * w and g are cast to fp8-e4m3 on the host (rel err 8.8e-3 vs the
    2e-2 budget) -> 8.4MB HBM traffic per core.
  * The matvec keeps g STATIONARY ([128,1] per k-tile, swapped 64x)
    and streams w as the MOVING operand (N=256 per matmul). The old
    design streamed g (N=1) with w stationary, paying the full
    isolated-matmul latency (~166ns) on every one of 512 weight swaps.
  * 4-way PE column tiling: each k-tile issues 4 concurrent matmuls in
    col-groups 0..3 (tile_position=(0,32c)), each covering a j-quarter
    of 256 outputs, accumulating into PSUM partitions 0/32/64/96. The
    four moving streams ride separate XBUSes, so w is ingested at up
    to 4 cols/cycle.
  * Host pre-arranges w per core as the exact SBUF image [128, 65536]
    (wt[p, t*1024+j] = w[t*128+p, j]) so every DMA chunk coalesces
    into 128 large per-partition descriptors instead of 8192 x 2KB.
  * Tail runs on 4 partition lanes (stride 32): ACT sigmoid, DVE
    mult/add with per-neuron B/D, ACT sigmoid, DMA out [4,256].
"""

from contextlib import ExitStack

import ml_dtypes
import numpy as np

import concourse.bass as bass
import concourse.bacc as bacc
import concourse.mybir as mybir
import concourse.tile as tile
from concourse.bass_utils import run_bass_kernel_spmd

N = 8192          # neurons
NCORES = 8
COLS = N // NCORES  # 1024 output neurons per core
P = 128           # partitions
KT = N // P       # 64 k-tiles (contraction)
GROUPS = 4        # PE column groups
JW = COLS // GROUPS  # 256 output cols per group
# DMA chunk sizes in k-tiles: small first chunks so the PE starts within
# ~0.5us of the first w byte; the rest sized for low per-dma_start cost.
# Taper both ends: small first chunks so the PE starts early, small last
# chunks so the PE finishes right behind the final DMA byte.
CHUNKS = [1, 2, 3, 4, 6, 8, 8, 8, 8, 8, 4, 3, 1]
assert sum(CHUNKS) == KT
SPIKE = 30.0
FP8 = ml_dtypes.float8_e4m3  # TRN float8e4 (max 240)

TRACE = False          # set True to capture NTFF profile
LAST_RESULT = None     # BassKernelResults of the most recent run

_NC = None


def _build(b_const):
    nc = bacc.Bacc("TRN2", target_bir_lowering=False, debug=False,
                   num_devices=NCORES)
    wt = nc.dram_tensor("wt", [P, KT * COLS], mybir.dt.float8e4,
                        kind="ExternalInput").ap()
    gt = nc.dram_tensor("gt", [P, KT], mybir.dt.float8e4,
                        kind="ExternalInput").ap()
    bd = nc.dram_tensor("bd", [GROUPS, JW], mybir.dt.float32,
                        kind="ExternalInput").ap()
    out = nc.dram_tensor("out", [GROUPS, JW], mybir.dt.float32,
                         kind="ExternalOutput").ap()

    with tile.TileContext(nc) as tc, ExitStack() as ctx:
        wpool = ctx.enter_context(tc.tile_pool(name="w", bufs=1))
        spool = ctx.enter_context(tc.tile_pool(name="s", bufs=1))
        ppool = ctx.enter_context(tc.tile_pool(name="p", bufs=1, space="PSUM"))

        # First w chunk is issued before g/bd so streaming starts ASAP.
        gsb = spool.tile([P, KT], mybir.dt.float8e4)
        bdsb = spool.tile([P, JW], mybir.dt.float32)
        acc = ppool.tile([P, JW], mybir.dt.float32)

        # ALL DMAs stay on the single sync HWDGE ring: completion-sem lanes
        # are assigned round-robin ACROSS rings, and cross-ring completion
        # order is not FIFO, so two rings sharing a lane can release a
        # waiter early (observed as a rare NaN). Same-ring DMAs complete in
        # FIFO order, which matches the lane tick order. gsb goes first
        # (needed by the first matmul, tiny); bd is only needed by the
        # tail, so its issue slot comes after all w chunks.
        nc.sync.dma_start(gsb[:], gt[:])
        wtiles = []
        k0 = 0
        for c, ct in enumerate(CHUNKS):
            wsb = wpool.tile([P, ct * COLS], mybir.dt.float8e4, tag=f"w{c}")
            nc.sync.dma_start(wsb[:], wt[:, k0 * COLS:(k0 + ct) * COLS])
            wtiles.append(wsb)
            k0 += ct
        nc.sync.dma_start(bdsb[0:P:P // GROUPS, :], bd[:])

        k0 = 0
        for c, ct in enumerate(CHUNKS):
            wsb = wtiles[c]
            for t in range(ct):
                kt = k0 + t
                for grp in range(GROUPS):
                    nc.tensor.matmul(
                        acc[32 * grp:32 * grp + 1, :],
                        gsb[:, kt:kt + 1],
                        wsb[:, t * COLS + grp * JW: t * COLS + (grp + 1) * JW],
                        start=(kt == 0),
                        stop=(kt == KT - 1),
                        tile_position=(0, 32 * grp),
                    )
            k0 += ct

        # Tail: out = sigmoid(B*sigmoid(acc*12/N) + D) with B constant
        # across neurons (E_L/tau_m are constant-filled), rewritten as
        # sigmoid(B*(I_sig + D/B)) so the only per-element operand is the
        # precomputed D/B vector (one DVE add per group). Engine partition
        # bases must be quadrant-aligned, so each group works at its PSUM
        # partition 32g; the final DMA gathers the strided lanes.
        # The ops run on ALL 128 partitions in one instruction each; rows
        # other than {0,32,64,96} compute garbage that is never read (the
        # final strided DMA picks only the 4 real lanes).
        # Two j-halves so the first half's output DMA (and its HBM write
        # receipt) overlaps the second half's compute.
        isig = spool.tile([P, JW], mybir.dt.float32)
        tmp = spool.tile([P, JW], mybir.dt.float32)
        res = spool.tile([P, JW], mybir.dt.float32)
        for h in range(2):
            cs = slice(h * JW // 2, (h + 1) * JW // 2)
            nc.scalar.activation(isig[:, cs], acc[:, cs],
                                 mybir.ActivationFunctionType.Sigmoid,
                                 scale=12.0 / N)
            nc.vector.tensor_add(tmp[:, cs], isig[:, cs], bdsb[:, cs])
            nc.scalar.activation(res[:, cs], tmp[:, cs],
                                 mybir.ActivationFunctionType.Sigmoid,
                                 scale=float(b_const))
            nc.sync.dma_start(out[:, cs], res[0:P:P // GROUPS, cs])
    nc.compile()
    return nc


def make_in_maps(x_in, v, g, w, E_L, tau_m):
    w8 = np.asarray(w, dtype=np.float32).astype(FP8)
    g8 = np.asarray(g, dtype=np.float32).astype(FP8)
    gt = np.ascontiguousarray(g8.reshape(KT, P).T)

    E = np.asarray(E_L, dtype=np.float64)
    TM = np.asarray(tau_m, dtype=np.float64)
    V = np.asarray(v, dtype=np.float64)
    X = np.asarray(x_in, dtype=np.float64)
    B = (SPIKE - E) / TM
    assert np.ptp(B) == 0.0, "kernel assumes per-neuron gain B is constant"
    b_const = float(B[0])
    DB = (V + (E - V) / TM - SPIKE + 0.9 * X * B) / b_const

    in_maps = []
    for c in range(NCORES):
        sl = slice(c * COLS, (c + 1) * COLS)
        # SBUF image: wt[p, t*COLS + j] = w8[t*128 + p, c*COLS + j]
        wtc = np.ascontiguousarray(
            w8[:, sl].reshape(KT, P, COLS).transpose(1, 0, 2).reshape(
                P, KT * COLS))
        in_maps.append({
            "wt": wtc,
            "gt": gt,
            "bd": np.ascontiguousarray(
                DB[sl].astype(np.float32).reshape(GROUPS, JW)),
        })
    return b_const, in_maps


def kernel(x_in, v, g, w, E_L, tau_m, tau_g=None, **_unused):
    global _NC, LAST_RESULT
    b_const, in_maps = make_in_maps(x_in, v, g, w, E_L, tau_m)
    if _NC is None:
        _NC = _build(b_const)
    LAST_RESULT = run_bass_kernel_spmd(_NC, in_maps, list(range(NCORES)),
                                       trace=TRACE)
    out = np.empty(N, dtype=np.float32)
    for c in range(NCORES):
        out[c * COLS:(c + 1) * COLS] = \
            LAST_RESULT.results[c]["out"].reshape(COLS)
    return out


# revision 24
# speedup vs baseline: 1.0660x; 1.0660x over previous
"""LIF neuron step on 8 Trainium2 NeuronCores.

Math (reference):
    I_raw   = g @ w                       # [N] vec-mat product, w is [N, N]
    I       = sigmoid(12/N * I_raw) + 0.9 * x_in
    v_next  = v + (E_L - v + I * (30 - E_L)) / tau_m
    out     = sigmoid(v_next - 30)

Everything after the matvec is affine in I_sig = sigmoid(12/N * I_raw):
    out = sigmoid(B * I_sig + D)
    B   = (30 - E_L) / tau_m
    D   = v + (E_L - v)/tau_m - 30 + 0.9 * x_in * B
B and D are tiny per-neuron vectors, computed on the host.

Sharding: w is split column-wise (output-neuron dim) into 8 shards of
[8192, 1024]; g is replicated. Each core computes its 1024 outputs fully
locally; host concatenates.

Design (v2, ~2.5x over the fp16 w-stationary version):
  * w and g are cast to fp8-e4m3 on the host (rel err 8.8e-3 vs the
    2e-2 budget) -> 8.4MB HBM traffic per core.
  * The matvec keeps g STATIONARY ([128,1] per k-tile, swapped 64x)
    and streams w as the MOVING operand (N=256 per matmul). The old
    design streamed g (N=1) with w stationary, paying the full
    isolated-matmul latency (~166ns) on every one of 512 weight swaps.
  * 4-way PE column tiling: each k-tile issues 4 concurrent matmuls in
    col-groups 0..3 (tile_position=(0,32c)), each covering a j-quarter
    of 256 outputs, accumulating into PSUM partitions 0/32/64/96. The
    four moving streams ride separate XBUSes, so w is ingested at up
    to 4 cols/cycle.
  * Host pre-arranges w per core as the exact SBUF image [128, 65536]
    (wt[p, t*1024+j] = w[t*128+p, j]) so every DMA chunk coalesces
    into 128 large per-partition descriptors instead of 8192 x 2KB.
  * Tail runs on 4 partition lanes (stride 32): ACT sigmoid, DVE
    mult/add with per-neuron B/D, ACT sigmoid, DMA out [4,256].
"""

from contextlib import ExitStack

import ml_dtypes
import numpy as np

import concourse.bass as bass
import concourse.bacc as bacc
import concourse.mybir as mybir
import concourse.tile as tile
from concourse.bass_utils import run_bass_kernel_spmd

N = 8192          # neurons
NCORES = 8
COLS = N // NCORES  # 1024 output neurons per core
P = 128           # partitions
KT = N // P       # 64 k-tiles (contraction)
GROUPS = 4        # PE column groups
JW = COLS // GROUPS  # 256 output cols per group
# DMA chunk sizes in k-tiles: small first chunks so the PE starts within
# ~0.5us of the first w byte; the rest sized for low per-dma_start cost.
# Taper both ends: small first chunks so the PE starts early, small last
# chunks so the PE finishes right behind the final DMA byte.
CHUNKS = [1, 2, 3, 4, 6, 8, 8, 8, 8, 8, 4, 3, 1]
assert sum(CHUNKS) == KT
SPIKE = 30.0
FP8 = ml_dtypes.float8_e4m3  # TRN float8e4 (max 240)

TRACE = False          # set True to capture NTFF profile
LAST_RESULT = None     # BassKernelResults of the most recent run

_NC = None


def _build(b_const):
    nc = bacc.Bacc("TRN2", target_bir_lowering=False, debug=False,
                   num_devices=NCORES)
    wt = nc.dram_tensor("wt", [P, KT * COLS], mybir.dt.float8e4,
                        kind="ExternalInput").ap()
    gt = nc.dram_tensor("gt", [P, KT], mybir.dt.float8e4,
                        kind="ExternalInput").ap()
    bd = nc.dram_tensor("bd", [GROUPS, JW], mybir.dt.float32,
                        kind="ExternalInput").ap()
    out = nc.dram_tensor("out", [GROUPS, JW], mybir.dt.float32,
                         kind="ExternalOutput").ap()

    with tile.TileContext(nc) as tc, ExitStack() as ctx:
        wpool = ctx.enter_context(tc.tile_pool(name="w", bufs=1))
        spool = ctx.enter_context(tc.tile_pool(name="s", bufs=1))
        ppool = ctx.enter_context(tc.tile_pool(name="p", bufs=1, space="PSUM"))

        # First w chunk is issued before g/bd so streaming starts ASAP.
        gsb = spool.tile([P, KT], mybir.dt.float8e4)
        bdsb = spool.tile([P, JW], mybir.dt.float32)
        acc = ppool.tile([P, JW], mybir.dt.float32)

        # ALL DMAs stay on the single sync HWDGE ring: completion-sem lanes
        # are assigned round-robin ACROSS rings, and cross-ring completion
        # order is not FIFO, so two rings sharing a lane can release a
        # waiter early (observed as a rare NaN). Same-ring DMAs complete in
        # FIFO order, which matches the lane tick order. gsb goes first
        # (needed by the first matmul, tiny); bd is only needed by the
        # tail, so its issue slot comes after all w chunks.
        nc.sync.dma_start(gsb[:], gt[:])
        wtiles = []
        k0 = 0
        for c, ct in enumerate(CHUNKS):
            wsb = wpool.tile([P, ct * COLS], mybir.dt.float8e4, tag=f"w{c}")
            nc.sync.dma_start(wsb[:], wt[:, k0 * COLS:(k0 + ct) * COLS])
            wtiles.append(wsb)
            k0 += ct
        nc.sync.dma_start(bdsb[0:P:P // GROUPS, :], bd[:])

        k0 = 0
        for c, ct in enumerate(CHUNKS):
            wsb = wtiles[c]
            for t in range(ct):
                kt = k0 + t
                for grp in range(GROUPS):
                    nc.tensor.matmul(
                        acc[32 * grp:32 * grp + 1, :],
                        gsb[:, kt:kt + 1],
                        wsb[:, t * COLS + grp * JW: t * COLS + (grp + 1) * JW],
                        start=(kt == 0),
                        stop=(kt == KT - 1),
                        tile_position=(0, 32 * grp),
                    )
            k0 += ct

        # Tail: out = sigmoid(B*sigmoid(acc*12/N) + D) with B constant
        # across neurons (E_L/tau_m are constant-filled), rewritten as
        # sigmoid(B*(I_sig + D/B)) so the only per-element operand is the
        # precomputed D/B vector (one DVE add per group). Engine partition
        # bases must be quadrant-aligned, so each group works at its PSUM
        # partition 32g; the final DMA gathers the strided lanes.
        # The ops run on ALL 128 partitions in one instruction each; rows
        # other than {0,32,64,96} compute garbage that is never read (the
        # final strided DMA picks only the 4 real lanes).
        isig = spool.tile([P, JW], mybir.dt.float32)
        tmp = spool.tile([P, JW], mybir.dt.float32)
        res = spool.tile([P, JW], mybir.dt.float32)
        nc.scalar.activation(isig[:, :], acc[:, :],
                             mybir.ActivationFunctionType.Sigmoid,
                             scale=12.0 / N)
        nc.vector.tensor_add(tmp[:, :], isig[:, :], bdsb[:, :])
        nc.scalar.activation(res[:, :], tmp[:, :],
                             mybir.ActivationFunctionType.Sigmoid,
                             scale=float(b_const))
        nc.sync.dma_start(out[:], res[0:P:P // GROUPS, :])
    nc.compile()
    return nc


def make_in_maps(x_in, v, g, w, E_L, tau_m):
    w8 = np.asarray(w, dtype=np.float32).astype(FP8)
    g8 = np.asarray(g, dtype=np.float32).astype(FP8)
    gt = np.ascontiguousarray(g8.reshape(KT, P).T)

    E = np.asarray(E_L, dtype=np.float64)
    TM = np.asarray(tau_m, dtype=np.float64)
    V = np.asarray(v, dtype=np.float64)
    X = np.asarray(x_in, dtype=np.float64)
    B = (SPIKE - E) / TM
    assert np.ptp(B) == 0.0, "kernel assumes per-neuron gain B is constant"
    b_const = float(B[0])
    DB = (V + (E - V) / TM - SPIKE + 0.9 * X * B) / b_const

    in_maps = []
    for c in range(NCORES):
        sl = slice(c * COLS, (c + 1) * COLS)
        # SBUF image: wt[p, t*COLS + j] = w8[t*128 + p, c*COLS + j]
        wtc = np.ascontiguousarray(
            w8[:, sl].reshape(KT, P, COLS).transpose(1, 0, 2).reshape(
                P, KT * COLS))
        in_maps.append({
            "wt": wtc,
            "gt": gt,
            "bd": np.ascontiguousarray(
                DB[sl].astype(np.float32).reshape(GROUPS, JW)),
        })
    return b_const, in_maps


def kernel(x_in, v, g, w, E_L, tau_m, tau_g=None, **_unused):
    global _NC, LAST_RESULT
    b_const, in_maps = make_in_maps(x_in, v, g, w, E_L, tau_m)
    if _NC is None:
        _NC = _build(b_const)
    LAST_RESULT = run_bass_kernel_spmd(_NC, in_maps, list(range(NCORES)),
                                       trace=TRACE)
    out = np.empty(N, dtype=np.float32)
    for c in range(NCORES):
        out[c * COLS:(c + 1) * COLS] = \
            LAST_RESULT.results[c]["out"].reshape(COLS)
    return out
